# revision 1
# baseline (speedup 1.0000x reference)
"""AdaptiveBlockSelfAttention Trainium2 kernel (8 NeuronCores).

Math (per batch b, channel c, in blocked layout):
  X_c = x[b,c] unfolded to a 256x256 matrix [n, p] (n = 16x16 block index,
        p = 16x16 pixel-in-block index).
  Q/K/V = per-pixel channel mixing (1x1 conv) of X across c.
  T = K^T Q  (contract n)            -> [q, p]   (= S^T of the reference)
  E = exp(T / sqrt(C))               (no max-subtraction; logits are small)
  U' = E^T @ [V | 1]                 -> [p, 0:256]=numerator, [p,256]=denom
  O = U'[:, :256] / U'[:, 256:]      rows of O are output blocks n'=p
  x1 = X + O ; out = x1 + FFN(x1)    FFN mixes channels per pixel.

Sharding: core k = (b = k//2, h = k%2).
  - Attention: core computes channels [h*96,(h+1)*96) over the full image
    (keeps all matmul free dims >= 256).
  - x1 exchanged between the pair with chunked 2-core AllGathers that
    overlap the attention phase.
  - FFN: core computes its half of the tokens (blocked order) with all 192
    channels; the token offset h*32768 arrives as a per-core input and is
    applied with dynamic (register) DMA offsets so the SPMD graph is
    identical on all cores.

dtypes: bf16 matmul operands and x1 spine (f32 PSUM accumulation, f32
output). Host-validated L2 rel err ~3e-3 (gate 2e-2).
"""
import os
os.environ.setdefault("MYCRO_LOCAL_CACHE", "1")
import numpy as np
import ml_dtypes
import concourse.bass as bass
import concourse.bacc as bacc
import concourse.tile as tile
import concourse.mybir as mybir
from concourse.bass_utils import run_bass_kernel_spmd

F32 = mybir.dt.float32
BF16 = mybir.dt.bfloat16
AF = mybir.ActivationFunctionType

B, C, H, W = 4, 192, 256, 256
NPIX = H * W            # 65536 tokens per batch
CH = C // 2             # 96 channels per core
HID = 384
TT = 512                # token tile
NPROJ = NPIX // TT      # 128 projection tiles
NFFN = (NPIX // 2) // TT  # 64 FFN tiles per core
SCALE = 1.0 / float(np.sqrt(C))
NCHUNK = 2              # AllGather chunks over channels
CCH = CH // NCHUNK

_NC_CACHE = {}


def build_nc(sim=False):
    nc = bacc.Bacc("TRN2", target_bir_lowering=False, debug=False,
                   num_devices=1 if sim else 8)
    x = nc.dram_tensor("x", [C + 1, NPIX], BF16, kind="ExternalInput")
    wq = nc.dram_tensor("wq", [C + 1, CH], BF16, kind="ExternalInput")
    wk = nc.dram_tensor("wk", [C + 1, CH], BF16, kind="ExternalInput")
    wv = nc.dram_tensor("wv", [C + 1, CH], BF16, kind="ExternalInput")
    wf1 = nc.dram_tensor("wf1", [C, HID], BF16, kind="ExternalInput")
    bf1c = nc.dram_tensor("bf1c", [HID, 1], F32, kind="ExternalInput")
    wf2 = nc.dram_tensor("wf2", [HID, C], BF16, kind="ExternalInput")
    bf2c = nc.dram_tensor("bf2c", [C, 1], F32, kind="ExternalInput")
    dyn = nc.dram_tensor("dyn", [1, 4], mybir.dt.uint32, kind="ExternalInput")
    out = nc.dram_tensor("out", [C, NPIX // 2], BF16, kind="ExternalOutput")

    # fused spill, pair-interleaved: [cp][j][u][i][n'][p]
    qkvs = nc.dram_tensor("qkvs", [CH // 2, 6 * NPIX], BF16)
    # x1s flat: block0 = peer-destined halves, block1 = own halves;
    # per-channel 32768-token half-images within each block
    x1s = nc.dram_tensor("x1s", [1, 2 * CH * (NPIX // 2)], BF16)
    # x1gp rows: [chunk g][rank r][cch]
    x1gp = nc.dram_tensor("x1gp", [NCHUNK * 2 * CCH, NPIX // 2], BF16)

    xa_v = x.ap()
    x3 = x.ap().rearrange("c (n p) -> c n p", p=256)      # residual view
    qkv_w = qkvs.ap().rearrange("cp (j u t) -> cp j u t", j=3, u=2)
    qkv_rP = qkvs.ap().rearrange("cp (j u i n p) -> cp n j u i p",
                                 j=3, u=2, i=2, p=256)
    x3i = x.ap().rearrange("c (i n p) -> c n i p", i=2, p=256)
    x1s_1 = x1s.ap()
    x1s_r = x1s.ap().rearrange("o (r t) -> (o r) t", t=NPIX // 2)
    x1gp_f = x1gp.ap()

    def with_track(a, off):
        return bass.AP(tensor=a.tensor, offset=a.offset, ap=a.ap,
                       const_val=a.const_val,
                       runtime_checks=a.runtime_checks,
                       dep_tracking_offset=off)
    x1s3 = x1s.ap().rearrange("c (n p) -> c n p", p=256)

    with tile.TileContext(nc) as tc:
        # ---- persistent weights ----
        with tc.tile_pool(name="wpool", bufs=1) as wp:
            w_a, w_b = {}, {}
            for nm, wt in (("q", wq), ("k", wk), ("v", wv)):
                w_a[nm] = wp.tile([128, CH], BF16, name=f"wa{nm}", tag=f"wa{nm}")
                w_b[nm] = wp.tile([C + 1 - 128, CH], BF16, name=f"wb{nm}",
                                  tag=f"wb{nm}")
                nc.sync.dma_start(w_a[nm][:], wt.ap()[0:128, :])
                nc.sync.dma_start(w_b[nm][:], wt.ap()[128:C + 1, :])
            wf1_a = wp.tile([CH, HID], BF16, name="wf1a", tag="wf1a")
            wf1_b = wp.tile([CH, HID], BF16, name="wf1b", tag="wf1b")
            nc.sync.dma_start(wf1_a[:], wf1.ap()[0:CH, :])
            nc.sync.dma_start(wf1_b[:], wf1.ap()[CH:C, :])
            wf2_h = []
            for hc in range(3):
                t = wp.tile([128, C], BF16, name=f"wf2{hc}", tag=f"wf2{hc}")
                nc.sync.dma_start(t[:], wf2.ap()[hc * 128:(hc + 1) * 128, :])
                wf2_h.append(t)
            bf1_t = []
            for hc in range(3):
                t = wp.tile([128, 1], F32, name=f"bf1{hc}", tag=f"bf1{hc}")
                nc.sync.dma_start(t[:], bf1c.ap()[hc * 128:(hc + 1) * 128, :])
                bf1_t.append(t)
            bf2_t = []
            for cc in range(2):
                t = wp.tile([CH, 1], F32, name=f"bf2{cc}", tag=f"bf2{cc}")
                nc.sync.dma_start(t[:], bf2c.ap()[cc * CH:(cc + 1) * CH, :])
                bf2_t.append(t)
            dyn_sb = wp.tile([1, 4], mybir.dt.uint32, name="dyn", tag="dyn")
            nc.sync.dma_start(dyn_sb[:], dyn.ap()[:, :])
            o_m = [nc.values_load(dyn_sb[0:1, i:i + 1], min_val=0,
                                  max_val=CH * (NPIX // 2),
                                  skip_runtime_bounds_check=True)
                   for i in range(2)]
            pb0 = nc.values_load(dyn_sb[0:1, 2:3], min_val=0, max_val=CCH,
                                 skip_runtime_bounds_check=True)
            tv = nc.values_load(dyn_sb[0:1, 3:4], min_val=0,
                                max_val=NPIX // 2,
                                skip_runtime_bounds_check=True)

            # ---- phase 1: QKV projections ----
            with tc.tile_pool(name="px", bufs=6) as px, \
                 tc.tile_pool(name="pev", bufs=4) as pev, \
                 tc.tile_pool(name="psP", bufs=8, space="PSUM") as psP:
                for t2 in range(NPROJ // 2):
                    t2sl = bass.ts(t2, 2 * TT)
                    xt0 = px.tile([128, 2 * TT], BF16, name="xt0", tag="xt0")
                    xt1 = px.tile([C + 1 - 128, 2 * TT], BF16, name="xt1",
                                  tag="xt1")
                    nc.sync.dma_start(xt0[:], xa_v[0:128, t2sl])
                    nc.scalar.dma_start(xt1[:], xa_v[128:C + 1, t2sl])
                    comb = pev.tile([CH, 6 * TT], BF16, name="comb",
                                    tag="comb")
                    for half in range(2):
                        hs = slice(half * TT, (half + 1) * TT)
                        for j, nm in enumerate(("q", "k", "v")):
                            ps = psP.tile([CH, TT], F32, name="pp", tag="pp")
                            nc.tensor.matmul(ps[:], w_a[nm][:], xt0[:, hs],
                                             start=True, stop=False)
                            nc.tensor.matmul(ps[:], w_b[nm][:], xt1[:, hs],
                                             start=False, stop=True)
                            dst = comb[:, (j * 2 + half) * TT:
                                       (j * 2 + half + 1) * TT]
                            if j == 1:
                                nc.scalar.copy(dst, ps[:])
                            else:
                                nc.vector.tensor_copy(dst, ps[:])
                    combv = comb[:].rearrange("c (j t) -> c j t", j=3)
                    for u in range(2):
                        weng = nc.sync if u == 0 else nc.gpsimd
                        weng.dma_start(
                            qkv_w[:, :, u, t2sl],
                            combv[u * 48:(u + 1) * 48, :, :])

            # ---- phase 2: per-channel attention (+ overlapped AllGather) ---
            with tc.tile_pool(name="aq", bufs=6) as aq, \
                 tc.tile_pool(name="ao", bufs=8) as ao, \
                 tc.tile_pool(name="ar", bufs=8) as ar, \
                 tc.tile_pool(name="psT", bufs=5, space="PSUM") as psT, \
                 tc.tile_pool(name="psU", bufs=3, space="PSUM") as psU:
                for cp in range(CH // 2):
                    c = 2 * cp
                    # pair tiles: 2 channels per DMA
                    # qkt2: (c2, j2{q,k}, i2, p256); vv2: (c2, i2, 257)
                    # qkt2 cols: (j{q,k}, u, i, p)
                    qkt2 = aq.tile([128, 2048], BF16, name="qkt2", tag="qkt2")
                    nc.sync.dma_start(
                        qkt2[:].rearrange("n (j u i p) -> n j u i p",
                                          j=2, u=2, p=256),
                        qkv_rP[cp, :, 0:2, :, :, :])
                    vv2 = aq.tile([128, 1028], BF16, name="vv2", tag="vv2")
                    vv2v = vv2[:].rearrange("n (u i p) -> n u i p", u=2, p=257)
                    nc.scalar.dma_start(vv2v[:, :, :, 0:256],
                                        qkv_rP[cp, :, 2, :, :, :])
                    nc.gpsimd.memset(vv2v[:, :, :, 256:257], 1.0)
                    for u in range(2):
                        esb = []
                        for j in range(2):
                            tps = psT.tile([128, 256], F32, name="t", tag="t")
                            for i in range(2):
                                kbase = 1024 + u * 512 + i * 256
                                nc.tensor.matmul(
                                    tps[:], qkt2[:, kbase + j * 128:
                                                 kbase + (j + 1) * 128],
                                    qkt2[:, u * 512 + i * 256:
                                         u * 512 + (i + 1) * 256],
                                    start=(i == 0), stop=(i == 1))
                            te = ar.tile([128, 256], BF16, name=f"e{j}",
                                         tag=f"e{j}")
                            nc.scalar.activation(te[:], tps[:], AF.Exp,
                                                 scale=SCALE)
                            esb.append(te)
                        ob2 = ao.tile([128, 512], BF16, name="ob2", tag="ob2")
                        for m in range(2):
                            msl = slice(m * 128, (m + 1) * 128)
                            ups = psU.tile([128, 257], F32, name="u", tag="u")
                            for i in range(2):
                                vsl = slice(u * 514 + i * 257,
                                            u * 514 + (i + 1) * 257)
                                nc.tensor.matmul(ups[:], esb[i][:, msl],
                                                 vv2[:, vsl],
                                                 start=(i == 0), stop=(i == 1))
                            rc = ar.tile([128, 1], F32, name="rc", tag="rc")
                            nc.vector.reciprocal(rc[:], ups[:, 256:257])
                            nc.vector.tensor_scalar_mul(
                                ob2[:, m * 256:(m + 1) * 256],
                                ups[:, 0:256], rc[:])
                        for m in range(2):
                            weng = nc.sync if m == 0 else nc.gpsimd
                            dst = x1s_1[0, bass.ds(
                                o_m[m] + (c + u) * (NPIX // 2), NPIX // 2)]
                            dst = with_track(dst, (c + u) * (NPIX // 2))
                            weng.dma_start(
                                dst, ob2[:, m * 256:(m + 1) * 256])
                    # chunked exchange as soon as a channel group is done
                    if (c + 2) % CCH == 0:
                        g = (c + 2) // CCH - 1
                        gsl = slice(g * CCH, (g + 1) * CCH)
                        src = x1s_r[gsl, :]
                        dst = x1gp_f[g * 2 * CCH:(g + 1) * 2 * CCH, :]
                        if sim:
                            dv = dst.rearrange("(r c) t -> r c t", r=2)
                            nc.sync.dma_start(dv[0], src)
                            nc.sync.dma_start(dv[1], src)
                        else:
                            nc.gpsimd.collective_compute(
                                "AllGather", mybir.AluOpType.bypass,
                                replica_groups=[[0, 1], [2, 3], [4, 5],
                                                [6, 7]],
                                ins=[src], outs=[dst],
                            )

            # ---- phase 3: FFN on my token half ----
            with tc.tile_pool(name="fx", bufs=4) as fx, \
                 tc.tile_pool(name="fh", bufs=4) as fh, \
                 tc.tile_pool(name="fo", bufs=4) as fo, \
                 tc.tile_pool(name="psH", bufs=5, space="PSUM") as psH, \
                 tc.tile_pool(name="psY", bufs=3, space="PSUM") as psY:
                for t in range(NFFN):
                    tsl = bass.ts(t, TT)
                    xf = []
                    to0 = fx.tile([CH, TT], BF16, name="to0", tag="to0")
                    src0 = with_track(x1s_r[CH:2 * CH, tsl], t * TT)
                    nc.sync.dma_start(to0[:], src0)
                    to1 = fx.tile([CH, TT], BF16, name="to1", tag="to1")
                    for g in range(NCHUNK):
                        eng = nc.gpsimd if g == 0 else nc.scalar
                        eng.dma_start(
                            to1[g * CCH:(g + 1) * CCH, :],
                            x1gp_f[bass.ds(pb0 + g * 2 * CCH, CCH), tsl])
                    for gr, to in ((0, to0), (1, to1)):
                        tx = fx.tile([CH, TT], BF16, name=f"txr{gr}",
                                     tag=f"txr{gr}")
                        eng = nc.sync if gr == 0 else nc.gpsimd
                        eng.dma_start(
                            tx[:], xa_v[gr * CH:(gr + 1) * CH,
                                        bass.ds(tv + t * TT, TT)])
                        tf = fx.tile([CH, TT], BF16, name=f"xf{gr}",
                                     tag=f"xf{gr}")
                        nc.vector.tensor_add(tf[:], to[:], tx[:])
                        xf.append(tf)
                    hsb = []
                    for hc in range(3):
                        hcs = slice(hc * 128, (hc + 1) * 128)
                        hps = psH.tile([128, TT], F32, name="h", tag="h")
                        nc.tensor.matmul(hps[:], wf1_a[:, hcs], xf[0][:],
                                         start=True, stop=False)
                        nc.tensor.matmul(hps[:], wf1_b[:, hcs], xf[1][:],
                                         start=False, stop=True)
                        th = fh.tile([128, TT], BF16, name=f"h{hc}",
                                     tag=f"h{hc}")
                        nc.scalar.activation(th[:], hps[:], AF.Gelu,
                                             bias=bf1_t[hc][:])
                        hsb.append(th)
                    for cc in range(2):
                        ccs = slice(cc * CH, (cc + 1) * CH)
                        yps = psY.tile([CH, TT], F32, name="y", tag="y")
                        for hc in range(3):
                            nc.tensor.matmul(yps[:], wf2_h[hc][:, ccs],
                                             hsb[hc][:], start=(hc == 0),
                                             stop=(hc == 2))
                        oo = fo.tile([CH, TT], BF16, name=f"oo{cc}",
                                     tag=f"oo{cc}")
                        nc.vector.tensor_add(oo[:], yps[:], xf[cc][:])
                        nc.vector.tensor_scalar_add(oo[:], oo[:],
                                                    bf2_t[cc][:])
                        eng = nc.sync if cc == 0 else nc.gpsimd
                        eng.dma_start(out.ap()[ccs, bass.ts(t, TT)], oo[:])
    nc.compile()
    return nc


def _get_nc():
    if "nc" not in _NC_CACHE:
        _NC_CACHE["nc"] = build_nc()
    return _NC_CACHE["nc"]


def _block(x):
    """(B,C,256,256) -> (B,C,65536) blocked token order."""
    Bn, Cn = x.shape[0], x.shape[1]
    return (x.reshape(Bn, Cn, 16, 16, 16, 16)
            .transpose(0, 1, 2, 4, 3, 5)
            .reshape(Bn, Cn, NPIX))


def _unblock(y):
    """(B,C,65536) blocked -> (B,C,256,256)."""
    Bn, Cn = y.shape[0], y.shape[1]
    return (y.reshape(Bn, Cn, 16, 16, 16, 16)
            .transpose(0, 1, 2, 4, 3, 5)
            .reshape(Bn, Cn, H, W))


def prepare_in_maps(x, Wq, bq, Wk, bk, Wv, bv, Wf1, bf1, Wf2, bf2):
    xb = _block(np.asarray(x, np.float32))
    xb_bf = xb.astype(ml_dtypes.bfloat16)
    ones = np.ones((1, NPIX), ml_dtypes.bfloat16)
    wf1_f = np.asarray(Wf1, np.float32)
    wf2_f = np.asarray(Wf2, np.float32)
    bf1_in = np.asarray(bf1, np.float32).reshape(HID, 1)
    bf2_f = np.asarray(bf2, np.float32)
    in_maps = []
    for k in range(8):
        b, h = k // 2, k % 2
        own = slice(h * CH, (h + 1) * CH)
        perm = np.r_[np.arange(h * CH, (h + 1) * CH),
                     np.arange((1 - h) * CH, (2 - h) * CH)]
        x_in = np.concatenate([xb_bf[b][perm], ones], axis=0)
        wf1_in = np.ascontiguousarray(wf1_f[:, perm].T
                                      ).astype(ml_dtypes.bfloat16)
        wf2_in = np.ascontiguousarray(wf2_f[perm].T
                                      ).astype(ml_dtypes.bfloat16)
        bf2_in = bf2_f[perm].reshape(C, 1)
        blk = CH * (NPIX // 2)
        dyn = np.array([[blk if h == 0 else 0, blk if h == 1 else 0,
                         (1 - h) * CCH, h * (NPIX // 2)]], np.uint32)
        m = {"x": np.ascontiguousarray(x_in), "dyn": dyn,
             "wf1": wf1_in, "wf2": wf2_in, "bf1c": bf1_in, "bf2c": bf2_in}
        eo = np.r_[np.arange(0, CH, 2), np.arange(1, CH, 2)]
        for nm, Wm, bm in (("wq", Wq, bq), ("wk", Wk, bk), ("wv", Wv, bv)):
            Wm = np.asarray(Wm, np.float32)
            wown = Wm[own][eo]        # spill order: evens then odds
            bown = np.asarray(bm, np.float32)[own][eo]
            wext = np.concatenate([wown[:, perm].T, bown[None, :]], axis=0)
            m[nm] = wext.astype(ml_dtypes.bfloat16)
        in_maps.append(m)
    return in_maps


def run(in_maps, trace=False, **kw):
    nc = _get_nc()
    return run_bass_kernel_spmd(nc, in_maps, core_ids=list(range(8)),
                                trace=trace, **kw)


def assemble(results):
    yb = np.empty((B, C, NPIX), np.float32)
    for k in range(8):
        b, h = k // 2, k % 2
        perm = np.r_[np.arange(h * CH, (h + 1) * CH),
                     np.arange((1 - h) * CH, (2 - h) * CH)]
        o = results[k]["out"]
        yb[b, perm, h * (NPIX // 2):(h + 1) * (NPIX // 2)] = \
            o.astype(np.float32)
    return _unblock(yb)


def kernel(**inputs):
    in_maps = prepare_in_maps(**inputs)
    res = run(in_maps)
    return assemble(res.results)

